# revision 17
# baseline (speedup 1.0000x reference)
"""Trainium2 Bass kernel for a 6-layer dense transformer (patch-embed ->
6x(MHA+FFN) -> token-predictor), sharded across 8 NeuronCores.

Sharding: the 4096 tokens (B=4 x N=1024) are split 8 ways: core c owns batch
element c//2, token half c%2 (512 tokens). Attention needs all 1024 keys of
the batch element, so each layer AllGathers the LN1 output y (fp8, 196KB)
between the two cores sharing a batch element; each core then recomputes the
partner-half K/V locally from the gathered y.

The partner-half selection (which half of the gathered buffer is "remote")
is core-dependent, but the SPMD program is shared — so the selection is baked
into per-core host-prepared weights: Wk_sel = concat(Wk*(1-hh), Wk*hh) along
the contraction dim. The remote K/V projection contracts over BOTH gathered
halves; the wrong half multiplies by zeros. Sel weights are host-scaled by
S=32 and stored fp8 (weights ~0.02 would be subnormal in e4m3); the psum ->
fp8 cast applies 1/S.

Attention is split local/remote: the local half (own K/V from full-precision
y) runs DURING the collective; the remote half runs after, accumulating into
the same softmax sums (no mask => key order irrelevant). Per-head local
partials are evicted to SBUF (bf16) to keep PSUM pressure at one bank.

LayerNorm rstd = exp(-0.5*ln(var+eps)) keeps Ln/Exp in one activation table
set (exp is also used by attention), so only Gelu forces table reloads
(2/layer instead of 4).

x / Wemb / Wp are bf16 (halves the dominant embed/unembed DMA; matmul speed
on TRN2 is dtype-independent at >=256-wide).
"""
import sys
import math

sys.path.insert(0, '/opt/trn_rl_repo')

import numpy as np
import ml_dtypes

B, NTOK, V, D, H, E, F, L = 4, 1024, 8192, 384, 6, 64, 1536, 6
NCORES = 8
T = NTOK * B // NCORES          # 512 tokens per core
TP, DP, FP, VP = T // 128, D // 128, F // 128, V // 128
EP = E + 1                      # head dim + denominator ones-column
YSZ = D * T                     # fp8 bytes of y in the collective bounce
XCH = 8                         # x k-tiles per DMA chunk (embed)
WCH = 8                         # Wemb k-tiles per DMA chunk
RG = [[0, 1], [2, 3], [4, 5], [6, 7]]
USE_F32R = True
SSEL = 32.0                     # host scale on fp8 selection weights


def _positional_encoding(n, d):
    position = np.arange(n)[:, None].astype(np.float32)
    div_term = np.exp(np.arange(0, d, 2).astype(np.float32)
                      * (-math.log(10000.0) / d))
    pe = np.zeros((n, d), dtype=np.float32)
    pe[:, 0::2] = np.sin(position * div_term)
    pe[:, 1::2] = np.cos(position * div_term)
    return pe


def build_nc():
    import concourse.bass as bass
    import concourse.mybir as mybir
    import concourse.tile as tile
    from concourse import bacc
    from concourse.bass import ts

    F32 = mybir.dt.float32
    F32R = mybir.dt.float32r if USE_F32R else F32
    BF16 = mybir.dt.bfloat16
    FP8 = mybir.dt.float8e4
    AF = mybir.ActivationFunctionType
    OP = mybir.AluOpType

    nc = bacc.Bacc("TRN2", target_bir_lowering=False, debug=False,
                   num_devices=NCORES, num_swdge_queues=4)

    # Steer the act-table picker: drop Exp/Ln from the single-function sets
    # so both resolve to natural_log_exp_and_others (which really contains
    # both on HW) — one shared table for LayerNorm's ln/exp and attention's
    # exp instead of alternating reloads.
    import concourse.hw_specs as hw_specs
    tabs = hw_specs.get_activation_tables(nc.m.arch)
    tabs["exp_and_others"].discard(AF.Exp)
    tabs["natural_log"].discard(AF.Ln)

    def dmaR(out, in_):
        # all operands are host-cast; plain HWDGE dma (SEQ released at
        # handoff, unlike Pool/SWDGE dmas which hold SEQ through transfer)
        return nc.sync.dma_start(out=out, in_=in_)

    xT = nc.dram_tensor("xT", [V, T], BF16, kind="ExternalInput")
    peb = nc.dram_tensor("peb", [T, D], F32, kind="ExternalInput")
    eye = nc.dram_tensor("eye", [128, 128], F32, kind="ExternalInput")
    onesv = nc.dram_tensor("onesv", [1, 128], BF16, kind="ExternalInput")
    Wemb = nc.dram_tensor("Wemb", [V, D], BF16, kind="ExternalInput")
    Wq = nc.dram_tensor("Wq", [L * D, D], BF16, kind="ExternalInput")
    Wk = nc.dram_tensor("Wk", [L * D, D], BF16, kind="ExternalInput")
    Wv = nc.dram_tensor("Wv", [L * D, D], BF16, kind="ExternalInput")
    Wo = nc.dram_tensor("Wo", [L * D, D], BF16, kind="ExternalInput")
    Wksel = nc.dram_tensor("Wksel", [L * 128, 6 * D], FP8,
                           kind="ExternalInput")
    Wvsel = nc.dram_tensor("Wvsel", [L * 128, 6 * D], FP8,
                           kind="ExternalInput")
    bqc = nc.dram_tensor("bqc", [L * D], F32, kind="ExternalInput")
    bkc = nc.dram_tensor("bkc", [L * D], F32, kind="ExternalInput")
    rows4 = nc.dram_tensor("rows4", [L, 4 * D], BF16, kind="ExternalInput")
    W1 = nc.dram_tensor("W1", [L * D, F], BF16, kind="ExternalInput")
    b1c = nc.dram_tensor("b1c", [L * F], F32, kind="ExternalInput")
    W2 = nc.dram_tensor("W2", [L * F, D], BF16, kind="ExternalInput")
    Wp = nc.dram_tensor("Wp", [D, V], BF16, kind="ExternalInput")
    bp = nc.dram_tensor("bp", [V], BF16, kind="ExternalInput")
    logits = nc.dram_tensor("logits", [T, V], F32, kind="ExternalOutput")

    with tile.TileContext(nc) as tc:
        import contextlib
        ctx = contextlib.ExitStack()
        # ---- persistent tiles ----
        singles = ctx.enter_context(tc.tile_pool(name="singles", bufs=1))
        eye_sb = singles.tile([128, 128], F32, name="eye_sb", tag="eye_sb")
        nc.sync.dma_start(eye_sb[:], eye.ap())
        ones_sb = singles.tile([1, 128], BF16, name="ones_sb", tag="ones_sb")
        dmaR(ones_sb[:], onesv.ap())
        ones_b16 = singles.tile([1, 128], BF16, name="ones_b16", tag="ones_b16")
        nc.vector.memset(ones_b16[:], 1.0)
        eps_sb = singles.tile([128, 1], F32, name="eps_sb", tag="eps_sb")
        nc.vector.memset(eps_sb[:], 1e-5)
        peb_sb = singles.tile([128, TP, D], F32, name="peb_sb", tag="peb_sb")
        nc.sync.dma_start(peb_sb[:],
                          peb.ap().rearrange("(m p) d -> p m d", p=128))
        resid = singles.tile([128, TP, D], F32, name="resid", tag="resid")

        # ---- PSUM pools (8 banks total) ----
        tmps = ctx.enter_context(tc.tile_pool(name="tmps", bufs=1, space="PSUM"))
        bigps = ctx.enter_context(tc.tile_pool(name="bigps", bufs=3, space="PSUM"))
        ops_ = ctx.enter_context(tc.tile_pool(name="ops", bufs=1, space="PSUM"))

        smallp = ctx.enter_context(tc.tile_pool(name="smallp", bufs=8))
        actp = ctx.enter_context(tc.tile_pool(name="actp", bufs=1))
        pp = ctx.enter_context(tc.tile_pool(name="pp", bufs=4))
        gp = ctx.enter_context(tc.tile_pool(name="gp", bufs=4))
        dramp = ctx.enter_context(tc.tile_pool(name="dramp", bufs=3, space="DRAM"))
        wpp = ctx.enter_context(tc.tile_pool(name="wpp", bufs=4))
        lgp = ctx.enter_context(tc.tile_pool(name="lgp", bufs=4))

        # ================= EMBED =================
        with tc.tile_pool(name="xp", bufs=2) as xp, \
             tc.tile_pool(name="wembp", bufs=2) as wep:
            emb_ps = [tmps.tile([128, D], F32, name=f"embps{mt}", tag=f"tm{mt}")
                      for mt in range(TP)]
            for ch in range(VP // XCH):
                x_t = xp.tile([128, XCH, T], BF16, name="x_t", tag="x")
                nc.sync.dma_start(
                    x_t[:], xT.ap()[ch * XCH * 128:(ch + 1) * XCH * 128, :]
                    .rearrange("(k p) n -> p k n", p=128))
                if XCH == WCH:
                    w_t = wep.tile([128, WCH, D], BF16, name="w_t", tag="wemb")
                    nc.sync.dma_start(
                        w_t[:], Wemb.ap()[ch * WCH * 128:(ch + 1) * WCH * 128, :]
                        .rearrange("(k p) n -> p k n", p=128))
                for k in range(XCH):
                    kt = ch * XCH + k
                    for mt in range(TP):
                        nc.tensor.matmul(emb_ps[mt][:],
                                         x_t[:, k, ts(mt, 128)],
                                         w_t[:, k, :],
                                         start=(kt == 0), stop=(kt == VP - 1))
            for mt in range(TP):
                nc.vector.tensor_add(resid[:, mt, :], emb_ps[mt][:],
                                     peb_sb[:, mt, :])

        # ---- shared activation tiles (allocated fresh per layer via pools) ----
        def layernorm(dst_tm, src):
            for mt in range(TP):
                stats = smallp.tile([128, 6], F32, name="stats", tag="stats")
                nc.vector.bn_stats(stats[:], src[:, mt, :])
                mv = smallp.tile([128, 2], F32, name="mv", tag="mv")
                nc.vector.bn_aggr(mv[:], stats[:])
                lnv = smallp.tile([128, 1], F32, name="lnv", tag="lnv")
                nc.scalar.activation(lnv[:], mv[:, 1:2], AF.Ln,
                                     bias=eps_sb[:, 0:1])
                rstd = smallp.tile([128, 1], F32, name="rstd", tag="rstd")
                nc.scalar.activation(rstd[:], lnv[:], AF.Exp, scale=-0.5)
                nc.vector.tensor_scalar(dst_tm[:, mt, :], src[:, mt, :],
                                        mv[:, 0:1], rstd[:],
                                        op0=OP.subtract, op1=OP.mult)

        def transpose_to(dst_fm, src_tm, dst8=None):
            for ft in range(DP):
                for mt in range(TP):
                    tp_ = bigps.tile([128, 128], F32, name="trps", tag="big")
                    nc.tensor.transpose(tp_[:], src_tm[:, mt, ts(ft, 128)],
                                        eye_sb[:])
                    nc.vector.tensor_copy(dst_fm[:, ft, ts(mt, 128)], tp_[:])
                    if dst8 is not None:
                        # Pool can't read PSUM; cast from the SBUF copy
                        nc.gpsimd.tensor_copy(dst8[:, ft, ts(mt, 128)],
                                              dst_fm[:, ft, ts(mt, 128)])

        # ================= LAYERS =================
        wp_pre = []
        with tc.tile_pool(name="wqkv", bufs=2) as wqkv, \
             tc.tile_pool(name="w1p", bufs=4) as w1p, \
             tc.tile_pool(name="w2p", bufs=3) as w2p, \
             tc.tile_pool(name="smallw", bufs=2) as smallw:
            for l in range(L):
                # --- LN1 + transpose (f32r + fp8 copies) ---
                y_tm = actp.tile([128, TP, D], F32, name="y_tm", tag="y_tm",
                                 bufs=1)
                layernorm(y_tm, resid)
                y_fm = actp.tile([128, DP, T], BF16, name="y_fm", tag="y_fm",
                                 bufs=2)
                y8 = actp.tile([128, DP, T], FP8, name="y8", tag="y8", bufs=2)
                transpose_to(y_fm, y_tm, dst8=y8)

                # --- y exchange: AllGather of fp8 y (starts ASAP) ---
                cc_in = dramp.tile([YSZ], FP8, name="cc_in", tag="cc_in")
                nc.sync.dma_start(
                    cc_in[:].rearrange("(t p n) -> p t n", p=128, n=T),
                    y8[:])
                cc_out = dramp.tile([2 * YSZ], FP8, name="cc_out",
                                    tag="cc_out")
                nc.gpsimd.collective_compute(
                    "AllGather", OP.bypass, replica_groups=RG,
                    ins=[cc_in[:].opt()], outs=[cc_out[:].opt()])

                if l == L - 1:
                    # prefetch all unembed weight chunks into the last
                    # layer's collective window
                    for c in range(4):
                        wp_t = wpp.tile([128, DP, 4 * T], BF16,
                                        name="wp_t", tag="wp")
                        nc.sync.dma_start(
                            wp_t[:], Wp.ap()[:, c * 4 * T:(c + 1) * 4 * T]
                            .rearrange("(t p) n -> p t n", p=128))
                        bp_t = wpp.tile([1, 4 * T], BF16, name="bp_t",
                                        tag="bp")
                        dmaR(bp_t[:], bp.ap()[c * 4 * T:(c + 1) * 4 * T]
                             .rearrange("(o n) -> o n", o=1))
                        wp_pre.append((wp_t, bp_t))

                # --- weight loads ---
                wq_sb = wqkv.tile([128, DP, D], BF16, name="wq_sb", tag="wq")
                dmaR(wq_sb[:], Wq.ap()[l * D:(l + 1) * D, :]
                     .rearrange("(k p) o -> p k o", p=128))
                wk_sb = wqkv.tile([128, DP, D], BF16, name="wk_sb", tag="wk")
                dmaR(wk_sb[:], Wk.ap()[l * D:(l + 1) * D, :]
                     .rearrange("(k p) o -> p k o", p=128))
                wv_sb = wqkv.tile([128, DP, D], BF16, name="wv_sb", tag="wv")
                dmaR(wv_sb[:], Wv.ap()[l * D:(l + 1) * D, :]
                     .rearrange("(k p) o -> p k o", p=128))
                wo_sb = wqkv.tile([128, DP, D], BF16, name="wo_sb", tag="wo")
                dmaR(wo_sb[:], Wo.ap()[l * D:(l + 1) * D, :]
                     .rearrange("(k p) o -> p k o", p=128))
                wksel8 = wqkv.tile([128, 2 * DP, D], FP8, name="wksel8",
                                   tag="wksel")
                nc.sync.dma_start(
                    wksel8[:], Wksel.ap()[l * 128:(l + 1) * 128, :]
                    .rearrange("p (k o) -> p k o", k=2 * DP))
                wvsel8 = wqkv.tile([128, 2 * DP, D], FP8, name="wvsel8",
                                   tag="wvsel")
                nc.sync.dma_start(
                    wvsel8[:], Wvsel.ap()[l * 128:(l + 1) * 128, :]
                    .rearrange("p (k o) -> p k o", k=2 * DP))
                bq_sb = smallw.tile([128, DP], F32, name="bq_sb", tag="bq")
                nc.sync.dma_start(bq_sb[:], bqc.ap()[l * D:(l + 1) * D]
                                  .rearrange("(t p) -> p t", p=128))
                bk_sb = smallw.tile([128, DP], F32, name="bk_sb", tag="bk")
                nc.sync.dma_start(bk_sb[:], bkc.ap()[l * D:(l + 1) * D]
                                  .rearrange("(t p) -> p t", p=128))
                r4_sb = smallw.tile([1, 4, D], BF16, name="r4_sb", tag="r4")
                dmaR(r4_sb[:], rows4.ap()[l:l + 1, :]
                     .rearrange("o (r d) -> o r d", r=4))
                b1_sb = smallw.tile([128, FP], F32, name="b1_sb", tag="b1")
                nc.sync.dma_start(b1_sb[:], b1c.ap()[l * F:(l + 1) * F]
                                  .rearrange("(t p) -> p t", p=128))

                # --- local K, V, Q projections (full-precision y) ---
                k_local = actp.tile([128, DP, T], FP8, name="k_local",
                                    tag="k_local")
                for t in range(DP):
                    psk = bigps.tile([128, T], F32, name="psk", tag="big")
                    for kt in range(DP):
                        nc.tensor.matmul(psk[:], wk_sb[:, kt, ts(t, 128)],
                                         y_fm[:, kt, :],
                                         start=(kt == 0), stop=(kt == DP - 1))
                    nc.scalar.activation(k_local[:, t, :], psk[:], AF.Identity,
                                         bias=bk_sb[:, t:t + 1])
                v_local = actp.tile([128, TP, H, EP], FP8, name="v_local",
                                    tag="v_local")
                nc.vector.memset(v_local[:, :, :, E:E + 1], 1.0)
                for mt in range(TP):
                    psv = bigps.tile([128, D], F32, name="psv", tag="big")
                    for kt in range(DP):
                        nc.tensor.matmul(psv[:], y_fm[:, kt, ts(mt, 128)],
                                         wv_sb[:, kt, :],
                                         start=(kt == 0), stop=False)
                    nc.tensor.matmul(psv[:], ones_sb[0:1, :],
                                     r4_sb[0:1, 0, :], start=False, stop=True)
                    nc.vector.tensor_copy(
                        v_local[:, mt, :, 0:E],
                        psv[:].rearrange("p (h e) -> p h e", h=H))
                q_fm = actp.tile([128, DP, T], FP8, name="q_fm", tag="q_fm")
                for t in range(DP):
                    psq = bigps.tile([128, T], F32, name="psq", tag="big")
                    for kt in range(DP):
                        nc.tensor.matmul(psq[:], wq_sb[:, kt, ts(t, 128)],
                                         y_fm[:, kt, :],
                                         start=(kt == 0), stop=(kt == DP - 1))
                    nc.vector.tensor_scalar_add(q_fm[:, t, :], psq[:],
                                                bq_sb[:, t:t + 1])

                # --- LOCAL attention (overlaps the collective) ---
                # head pairs run interleaved on two PSUM banks so the ACT
                # exps of one head hide behind the other head's matmuls
                o_loc = [actp.tile([EP, T], BF16, name=f"o_loc{h}",
                                   tag=f"o_loc{h}") for h in range(H)]
                for hp in range(H // 2):
                    h0, h1 = 2 * hp, 2 * hp + 1
                    ps = []
                    ps.append(ops_.tile([EP, T], F32, name="o_ps", tag="o"))
                    ps.append(tmps.tile([EP, T], F32, name="o_ps3",
                                        tag="tm3"))
                    for m in range(TP):
                        pts = []
                        for i, h in enumerate((h0, h1)):
                            po, pt = (h % 2) * E, h // 2
                            sc = bigps.tile([128, T], F32, name="sc",
                                            tag="big")
                            nc.tensor.matmul(
                                sc[:], k_local[po:po + E, pt, ts(m, 128)],
                                q_fm[po:po + E, pt, :],
                                start=True, stop=True)
                            p_t = pp.tile([128, T], FP8, name="p_t", tag="p")
                            nc.scalar.activation(p_t[:], sc[:], AF.Exp)
                            pts.append(p_t)
                        for i, h in enumerate((h0, h1)):
                            nc.tensor.matmul(ps[i][:], v_local[:, m, h, :],
                                             pts[i][:], start=(m == 0),
                                             stop=(m == TP - 1))
                    nc.vector.tensor_copy(o_loc[h0][:], ps[0][:])
                    nc.vector.tensor_copy(o_loc[h1][:], ps[1][:])

                # --- keep-warm: PE runs dry ~9us before the collective
                # lands; idle >3us drops the modeled PE clock to mid-state
                # for the first 3us of the remote phase. A chained block of
                # throwaway matmuls pins the ramp (and fills the gap).
                warm_ps = bigps.tile([128, T], F32, name="warm", tag="big")
                for w in range(40):
                    nc.tensor.matmul(warm_ps[:], wk_sb[:, 0, ts(0, 128)],
                                     y_fm[:, 0, :], start=True, stop=True)

                # --- gathered y in (both halves) ---
                yg = actp.tile([128, 2 * DP, T], FP8, name="yg", tag="yg")
                for r in range(2):
                    nc.sync.dma_start(
                        yg[:, r * DP:(r + 1) * DP, :],
                        cc_out[r * YSZ:(r + 1) * YSZ]
                        .rearrange("(t p n) -> p t n", p=128, n=T))

                # --- remote K/V via per-core selection weights ---
                k_rem = actp.tile([128, DP, T], FP8, name="k_rem",
                                  tag="k_rem")
                for t in range(DP):
                    pskr = bigps.tile([128, T], F32, name="pskr", tag="big")
                    for kt in range(2 * DP):
                        nc.tensor.matmul(pskr[:], wksel8[:, kt, ts(t, 128)],
                                         yg[:, kt, :],
                                         start=(kt == 0),
                                         stop=(kt == 2 * DP - 1))
                    nc.scalar.activation(k_rem[:, t, :], pskr[:], AF.Identity,
                                         bias=bk_sb[:, t:t + 1],
                                         scale=1.0 / SSEL)
                v_rem = actp.tile([128, TP, H, EP], FP8, name="v_rem",
                                  tag="v_rem")
                nc.vector.memset(v_rem[:, :, :, E:E + 1], 1.0)
                for mt in range(TP):
                    psvr = bigps.tile([128, D], F32, name="psvr", tag="big")
                    for kt in range(2 * DP):
                        nc.tensor.matmul(psvr[:], yg[:, kt, ts(mt, 128)],
                                         wvsel8[:, kt, :],
                                         start=(kt == 0), stop=False)
                    nc.tensor.matmul(psvr[:], ones_sb[0:1, :],
                                     r4_sb[0:1, 3, :], start=False, stop=True)
                    nc.scalar.activation(
                        v_rem[:, mt, :, 0:E],
                        psvr[:].rearrange("p (h e) -> p h e", h=H),
                        AF.Copy, scale=1.0 / SSEL)

                # --- REMOTE attention + merge ---
                # o_ps alternates between two banks (tm3 is attention-idle)
                # and each head's merge is emitted one head late, so the
                # merge's DVE chain never head-of-line blocks the PE queue.
                o_fm = actp.tile([128, DP, T], BF16, name="o_fm", tag="o_fm")

                def merge_head(h, o_ps):
                    po, pt = (h % 2) * E, h // 2
                    den_f = smallp.tile([1, T], F32, name="den_f",
                                        tag="den_f", bufs=2)
                    nc.vector.tensor_add(den_f[:], o_loc[h][E:E + 1, :],
                                         o_ps[E:E + 1, :])
                    recip_f = smallp.tile([1, T], F32, name="recip_f",
                                          tag="recip_f", bufs=2)
                    nc.vector.reciprocal(recip_f[:], den_f[:])
                    recip_r = smallp.tile([1, T], BF16, name="recip_r",
                                          tag="recip_r", bufs=2)
                    nc.scalar.activation(recip_r[:], recip_f[:], AF.Copy)
                    bc_ps = bigps.tile([E, T], F32, name="bc_ps", tag="big")
                    nc.tensor.matmul(bc_ps[:], ones_b16[0:1, 0:E], recip_r[:],
                                     start=True, stop=True)
                    bc_sb = pp.tile([E, T], F32, name="bc_sb", tag="bc",
                                    bufs=2)
                    nc.vector.tensor_copy(bc_sb[:], bc_ps[:])
                    o_t = pp.tile([E, T], F32, name="o_t", tag="o_t", bufs=2)
                    nc.vector.tensor_add(o_t[:], o_loc[h][0:E, :],
                                         o_ps[0:E, :])
                    nc.vector.tensor_mul(o_fm[po:po + E, pt, :],
                                         o_t[:], bc_sb[:])

                prev_pair = None
                for hp in range(H // 2):
                    h0, h1 = 2 * hp, 2 * hp + 1
                    ps = []
                    ps.append(ops_.tile([EP, T], F32, name="o_ps", tag="o"))
                    ps.append(tmps.tile([EP, T], F32, name="o_ps3",
                                        tag="tm3"))
                    for m in range(TP):
                        pts = []
                        for h in (h0, h1):
                            po, pt = (h % 2) * E, h // 2
                            sc = bigps.tile([128, T], F32, name="sc",
                                            tag="big")
                            nc.tensor.matmul(
                                sc[:], k_rem[po:po + E, pt, ts(m, 128)],
                                q_fm[po:po + E, pt, :],
                                start=True, stop=True)
                            p_t = pp.tile([128, T], FP8, name="p_t", tag="p")
                            nc.scalar.activation(p_t[:], sc[:], AF.Exp)
                            pts.append(p_t)
                        for i, h in enumerate((h0, h1)):
                            nc.tensor.matmul(ps[i][:], v_rem[:, m, h, :],
                                             pts[i][:], start=(m == 0),
                                             stop=(m == TP - 1))
                    if prev_pair is not None:
                        merge_head(prev_pair[0], prev_pair[1])
                        merge_head(prev_pair[2], prev_pair[3])
                    prev_pair = (h0, ps[0], h1, ps[1])
                merge_head(prev_pair[0], prev_pair[1])
                merge_head(prev_pair[2], prev_pair[3])

                # --- Wo + residual ---
                for mt in range(TP):
                    pso = tmps.tile([128, D], F32, name="pso", tag=f"tm{mt}")
                    for kt in range(DP):
                        nc.tensor.matmul(pso[:], o_fm[:, kt, ts(mt, 128)],
                                         wo_sb[:, kt, :],
                                         start=(kt == 0), stop=False)
                    nc.tensor.matmul(pso[:], ones_sb[0:1, :],
                                     r4_sb[0:1, 1, :], start=False, stop=True)
                    nc.vector.tensor_add(resid[:, mt, :], resid[:, mt, :],
                                         pso[:])

                # --- LN2 + transpose ---
                y_tm2 = actp.tile([128, TP, D], F32, name="y_tm2", tag="y_tm")
                layernorm(y_tm2, resid)
                y2_fm = actp.tile([128, DP, T], BF16, name="y2_fm",
                                  tag="y_fm", bufs=2)
                transpose_to(y2_fm, y_tm2)

                # --- FFN (streamed: FFN1 tile -> gelu -> FFN2 partial) ---
                w1_t = []
                for kt in range(DP):
                    w1k = w1p.tile([128, F], BF16, name="w1k", tag="w1")
                    dmaR(w1k[:], W1.ap()[l * D + kt * 128:
                                         l * D + (kt + 1) * 128, :])
                    w1_t.append(w1k)
                f2ps = [tmps.tile([128, D], F32, name=f"f2ps{mt}",
                                  tag=f"tm{mt}") for mt in range(TP)]
                for ft in range(FP):
                    if ft % 4 == 0:
                        w2c = w2p.tile([128, 4, D], BF16, name="w2c", tag="w2")
                        dmaR(w2c[:], W2.ap()[l * F + (ft // 4) * 512:
                                             l * F + (ft // 4 + 1) * 512, :]
                             .rearrange("(k p) o -> p k o", p=128))
                    psf = bigps.tile([128, T], F32, name="psf", tag="big")
                    for kt in range(DP):
                        nc.tensor.matmul(psf[:], w1_t[kt][:, ts(ft, 128)],
                                         y2_fm[:, kt, :],
                                         start=(kt == 0), stop=(kt == DP - 1))
                    g_t = gp.tile([128, T], BF16, name="g_t", tag="g")
                    nc.scalar.activation(g_t[:], psf[:], AF.Gelu,
                                         bias=b1_sb[:, ft:ft + 1])
                    for mt in range(TP):
                        nc.tensor.matmul(f2ps[mt][:], g_t[:, ts(mt, 128)],
                                         w2c[:, ft % 4, :],
                                         start=(ft == 0), stop=False)
                for mt in range(TP):
                    nc.tensor.matmul(f2ps[mt][:], ones_sb[0:1, :],
                                     r4_sb[0:1, 2, :], start=False, stop=True)
                    nc.vector.tensor_add(resid[:, mt, :], resid[:, mt, :],
                                         f2ps[mt][:])

        # ================= FINAL LN + UNEMBED =================
        lnf_tm = actp.tile([128, TP, D], F32, name="lnf_tm", tag="y_tm")
        layernorm(lnf_tm, resid)
        lnf_fm = actp.tile([128, DP, T], BF16, name="lnf_fm", tag="lnf_fm")
        for ft in range(DP):
            for mt in range(TP):
                tp_ = bigps.tile([128, 128], F32, name="trps", tag="big")
                nc.tensor.transpose(tp_[:], lnf_tm[:, mt, ts(ft, 128)],
                                    eye_sb[:])
                nc.vector.tensor_copy(lnf_fm[:, ft, ts(mt, 128)], tp_[:])

        if True:
            for c in range(V // (4 * T)):
                if c < len(wp_pre):
                    wp_t, bp_t = wp_pre[c]
                else:
                    wp_t = wpp.tile([128, DP, 4 * T], BF16, name="wp_t",
                                    tag="wp")
                    nc.sync.dma_start(
                        wp_t[:], Wp.ap()[:, c * 4 * T:(c + 1) * 4 * T]
                        .rearrange("(t p) n -> p t n", p=128))
                    bp_t = wpp.tile([1, 4 * T], BF16, name="bp_t", tag="bp")
                    dmaR(bp_t[:], bp.ap()[c * 4 * T:(c + 1) * 4 * T]
                         .rearrange("(o n) -> o n", o=1))
                for half in range(4):
                    vc = c * 4 + half
                    for mt in range(TP):
                        psl = bigps.tile([128, T], F32, name="psl", tag="big")
                        for kt in range(DP):
                            nc.tensor.matmul(psl[:],
                                             lnf_fm[:, kt, ts(mt, 128)],
                                             wp_t[:, kt,
                                                  half * T:(half + 1) * T],
                                             start=(kt == 0), stop=False)
                        nc.tensor.matmul(psl[:], ones_sb[0:1, :],
                                         bp_t[0:1, half * T:(half + 1) * T],
                                         start=False, stop=True)
                        lg = lgp.tile([128, T], F32, name="lg", tag="lg")
                        if (vc * TP + mt) % 2 == 0:
                            nc.vector.tensor_copy(lg[:], psl[:])
                        else:
                            nc.scalar.activation(lg[:], psl[:], AF.Copy)
                        nc.sync.dma_start(
                            logits.ap()[mt * 128:(mt + 1) * 128,
                                        vc * T:(vc + 1) * T], lg[:])
        ctx.close()

    nc.compile()
    return nc


def _prep_inputs(inputs):
    f = {k: np.asarray(v, dtype=np.float32) for k, v in inputs.items()}
    x, Wemb_, bemb = f["x"], f["Wemb"], f["bemb"]
    scale = E ** -0.5
    Wq_p = np.empty((L, D, D), np.float32)
    Wk_p = np.empty((L, D, D), np.float32)
    Wv_p = np.empty((L, D, D), np.float32)
    bq_p = np.empty((L, D), np.float32)
    bk_p = np.empty((L, D), np.float32)
    rows4 = np.empty((L, 4, D), np.float32)
    W1_p = np.empty((L, D, F), np.float32)
    b1_p = np.empty((L, F), np.float32)
    for l in range(L):
        g1, b1l = f["ln1_g"][l], f["ln1_b"][l]
        Wq_l = f["Wq"][l].transpose(1, 0, 2).reshape(D, D)
        Wk_l = f["Wk"][l].transpose(1, 0, 2).reshape(D, D)
        Wv_l = f["Wv"][l].transpose(1, 0, 2).reshape(D, D)
        Wq_p[l] = (g1[:, None] * Wq_l) * scale
        bq_p[l] = (b1l @ Wq_l + f["bq"][l].reshape(-1)) * scale
        Wk_p[l] = g1[:, None] * Wk_l
        bk_p[l] = b1l @ Wk_l + f["bk"][l].reshape(-1)
        Wv_p[l] = g1[:, None] * Wv_l
        rows4[l, 0] = b1l @ Wv_l + f["bv"][l].reshape(-1)
        rows4[l, 1] = f["bo"][l]
        g2, b2l = f["ln2_g"][l], f["ln2_b"][l]
        W1_p[l] = g2[:, None] * f["W1"][l]
        b1_p[l] = b2l @ f["W1"][l] + f["b1"][l]
        rows4[l, 2] = f["b2"][l]
        rows4[l, 3] = SSEL * rows4[l, 0]
    Wp_p = f["lnf_g"][:, None] * f["Wp"]
    bp_p = f["lnf_b"] @ f["Wp"] + f["bp"]
    pe = _positional_encoding(NTOK, D)

    bf16 = ml_dtypes.bfloat16
    fp8 = ml_dtypes.float8_e4m3
    shared = {
        "eye": np.eye(128, dtype=np.float32),
        "onesv": np.ones((1, 128), bf16),
        "Wemb": np.ascontiguousarray(Wemb_.astype(bf16)),
        "Wq": np.ascontiguousarray(Wq_p.reshape(L * D, D).astype(bf16)),
        "Wk": np.ascontiguousarray(Wk_p.reshape(L * D, D).astype(bf16)),
        "Wv": np.ascontiguousarray(Wv_p.reshape(L * D, D).astype(bf16)),
        "Wo": np.ascontiguousarray(f["Wo"].reshape(L * D, D).astype(bf16)),
        "bqc": np.ascontiguousarray(bq_p.reshape(L * D)),
        "bkc": np.ascontiguousarray(bk_p.reshape(L * D)),
        "rows4": np.ascontiguousarray(rows4.reshape(L, 4 * D).astype(bf16)),
        "W1": np.ascontiguousarray(W1_p.reshape(L * D, F).astype(bf16)),
        "b1c": np.ascontiguousarray(b1_p.reshape(L * F)),
        "W2": np.ascontiguousarray(f["W2"].reshape(L * F, D).astype(bf16)),
        "Wp": np.ascontiguousarray(Wp_p.astype(bf16)),
        "bp": np.ascontiguousarray(bp_p.astype(bf16)),
    }
    in_maps = []
    for c in range(NCORES):
        bb, hh = c // 2, c % 2
        n0 = hh * T
        m = dict(shared)
        m["xT"] = np.ascontiguousarray(x[bb, n0:n0 + T, :].T.astype(bf16))
        m["peb"] = np.ascontiguousarray(pe[n0:n0 + T] + bemb)
        ksel = np.concatenate([Wk_p * (SSEL * hh), Wk_p * (SSEL * (1 - hh))],
                              axis=1)
        vsel = np.concatenate([Wv_p * (SSEL * hh), Wv_p * (SSEL * (1 - hh))],
                              axis=1)
        # per-partition-contiguous layout: [L,128,6*D] so each partition's
        # 2304B arrives as one descriptor (<512B elems pay 2x in the DMA)
        m["Wksel"] = np.ascontiguousarray(
            ksel.reshape(L, 6, 128, D).transpose(0, 2, 1, 3)
            .reshape(L * 128, 6 * D).astype(fp8))
        m["Wvsel"] = np.ascontiguousarray(
            vsel.reshape(L, 6, 128, D).transpose(0, 2, 1, 3)
            .reshape(L * 128, 6 * D).astype(fp8))
        in_maps.append(m)
    return in_maps


_NC_CACHE = []


def kernel(**inputs):
    import time
    from concourse.bass_utils import run_bass_kernel_spmd

    in_maps = _prep_inputs(inputs)
    if not _NC_CACHE:
        _NC_CACHE.append(build_nc())
    nc = _NC_CACHE[0]
    t0 = time.time()
    res = run_bass_kernel_spmd(nc, in_maps, core_ids=list(range(NCORES)))
    t1 = time.time()
    print(f"[kernel] run_bass_kernel_spmd wall: {(t1 - t0) * 1e3:.1f} ms",
          file=sys.stderr)
    out = np.empty((B, NTOK, V), np.float32)
    for c in range(NCORES):
        out[c // 2, (c % 2) * T:(c % 2) * T + T, :] = res.results[c]["logits"]
    return out


# revision 18
# speedup vs baseline: 1.0052x; 1.0052x over previous
"""Trainium2 Bass kernel for a 6-layer dense transformer (patch-embed ->
6x(MHA+FFN) -> token-predictor), sharded across 8 NeuronCores.

Sharding: the 4096 tokens (B=4 x N=1024) are split 8 ways: core c owns batch
element c//2, token half c%2 (512 tokens). Attention needs all 1024 keys of
the batch element, so each layer AllGathers the LN1 output y (fp8, 196KB)
between the two cores sharing a batch element; each core then recomputes the
partner-half K/V locally from the gathered y.

The partner-half selection (which half of the gathered buffer is "remote")
is core-dependent, but the SPMD program is shared — so the selection is baked
into per-core host-prepared weights: Wk_sel = concat(Wk*(1-hh), Wk*hh) along
the contraction dim. The remote K/V projection contracts over BOTH gathered
halves; the wrong half multiplies by zeros. Sel weights are host-scaled by
S=32 and stored fp8 (weights ~0.02 would be subnormal in e4m3); the psum ->
fp8 cast applies 1/S.

Attention is split local/remote: the local half (own K/V from full-precision
y) runs DURING the collective; the remote half runs after, accumulating into
the same softmax sums (no mask => key order irrelevant). Per-head local
partials are evicted to SBUF (bf16) to keep PSUM pressure at one bank.

LayerNorm rstd = exp(-0.5*ln(var+eps)) keeps Ln/Exp in one activation table
set (exp is also used by attention), so only Gelu forces table reloads
(2/layer instead of 4).

x / Wemb / Wp are bf16 (halves the dominant embed/unembed DMA; matmul speed
on TRN2 is dtype-independent at >=256-wide).
"""
import sys
import math

sys.path.insert(0, '/opt/trn_rl_repo')

import numpy as np
import ml_dtypes

B, NTOK, V, D, H, E, F, L = 4, 1024, 8192, 384, 6, 64, 1536, 6
NCORES = 8
T = NTOK * B // NCORES          # 512 tokens per core
TP, DP, FP, VP = T // 128, D // 128, F // 128, V // 128
EP = E + 1                      # head dim + denominator ones-column
YSZ = D * T                     # fp8 bytes of y in the collective bounce
XCH = 8                         # x k-tiles per DMA chunk (embed)
WCH = 8                         # Wemb k-tiles per DMA chunk
RG = [[0, 1], [2, 3], [4, 5], [6, 7]]
USE_F32R = True
SSEL = 32.0                     # host scale on fp8 selection weights


def _positional_encoding(n, d):
    position = np.arange(n)[:, None].astype(np.float32)
    div_term = np.exp(np.arange(0, d, 2).astype(np.float32)
                      * (-math.log(10000.0) / d))
    pe = np.zeros((n, d), dtype=np.float32)
    pe[:, 0::2] = np.sin(position * div_term)
    pe[:, 1::2] = np.cos(position * div_term)
    return pe


def build_nc():
    import concourse.bass as bass
    import concourse.mybir as mybir
    import concourse.tile as tile
    from concourse import bacc
    from concourse.bass import ts

    F32 = mybir.dt.float32
    F32R = mybir.dt.float32r if USE_F32R else F32
    BF16 = mybir.dt.bfloat16
    FP8 = mybir.dt.float8e4
    AF = mybir.ActivationFunctionType
    OP = mybir.AluOpType

    nc = bacc.Bacc("TRN2", target_bir_lowering=False, debug=False,
                   num_devices=NCORES, num_swdge_queues=4)

    # Steer the act-table picker: drop Exp/Ln from the single-function sets
    # so both resolve to natural_log_exp_and_others (which really contains
    # both on HW) — one shared table for LayerNorm's ln/exp and attention's
    # exp instead of alternating reloads.
    import concourse.hw_specs as hw_specs
    tabs = hw_specs.get_activation_tables(nc.m.arch)
    tabs["exp_and_others"].discard(AF.Exp)
    tabs["natural_log"].discard(AF.Ln)

    def dmaR(out, in_):
        # all operands are host-cast; plain HWDGE dma (SEQ released at
        # handoff, unlike Pool/SWDGE dmas which hold SEQ through transfer)
        return nc.sync.dma_start(out=out, in_=in_)

    xT = nc.dram_tensor("xT", [V, T], BF16, kind="ExternalInput")
    peb = nc.dram_tensor("peb", [T, D], F32, kind="ExternalInput")
    eye = nc.dram_tensor("eye", [128, 128], F32, kind="ExternalInput")
    onesv = nc.dram_tensor("onesv", [1, 128], BF16, kind="ExternalInput")
    Wemb = nc.dram_tensor("Wemb", [V, D], BF16, kind="ExternalInput")
    Wq = nc.dram_tensor("Wq", [L * D, D], BF16, kind="ExternalInput")
    Wk = nc.dram_tensor("Wk", [L * D, D], BF16, kind="ExternalInput")
    Wv = nc.dram_tensor("Wv", [L * D, D], BF16, kind="ExternalInput")
    Wo = nc.dram_tensor("Wo", [L * D, D], BF16, kind="ExternalInput")
    Wksel = nc.dram_tensor("Wksel", [L * 128, 6 * D], FP8,
                           kind="ExternalInput")
    Wvsel = nc.dram_tensor("Wvsel", [L * 128, 6 * D], FP8,
                           kind="ExternalInput")
    bqc = nc.dram_tensor("bqc", [L * D], F32, kind="ExternalInput")
    bkc = nc.dram_tensor("bkc", [L * D], F32, kind="ExternalInput")
    rows4 = nc.dram_tensor("rows4", [L, 4 * D], BF16, kind="ExternalInput")
    W1 = nc.dram_tensor("W1", [L * D, F], BF16, kind="ExternalInput")
    b1c = nc.dram_tensor("b1c", [L * F], F32, kind="ExternalInput")
    W2 = nc.dram_tensor("W2", [L * F, D], BF16, kind="ExternalInput")
    Wp = nc.dram_tensor("Wp", [D, V], BF16, kind="ExternalInput")
    bp = nc.dram_tensor("bp", [V], BF16, kind="ExternalInput")
    logits = nc.dram_tensor("logits", [T, V], F32, kind="ExternalOutput")

    with tile.TileContext(nc) as tc:
        import contextlib
        ctx = contextlib.ExitStack()
        # ---- persistent tiles ----
        singles = ctx.enter_context(tc.tile_pool(name="singles", bufs=1))
        eye_sb = singles.tile([128, 128], F32, name="eye_sb", tag="eye_sb")
        nc.sync.dma_start(eye_sb[:], eye.ap())
        ones_sb = singles.tile([1, 128], BF16, name="ones_sb", tag="ones_sb")
        dmaR(ones_sb[:], onesv.ap())
        ones_b16 = singles.tile([1, 128], BF16, name="ones_b16", tag="ones_b16")
        nc.vector.memset(ones_b16[:], 1.0)
        eps_sb = singles.tile([128, 1], F32, name="eps_sb", tag="eps_sb")
        nc.vector.memset(eps_sb[:], 1e-5)
        peb_sb = singles.tile([128, TP, D], F32, name="peb_sb", tag="peb_sb")
        nc.sync.dma_start(peb_sb[:],
                          peb.ap().rearrange("(m p) d -> p m d", p=128))
        resid = singles.tile([128, TP, D], F32, name="resid", tag="resid")

        # ---- PSUM pools (8 banks total) ----
        tmps = ctx.enter_context(tc.tile_pool(name="tmps", bufs=1, space="PSUM"))
        bigps = ctx.enter_context(tc.tile_pool(name="bigps", bufs=3, space="PSUM"))
        ops_ = ctx.enter_context(tc.tile_pool(name="ops", bufs=1, space="PSUM"))

        smallp = ctx.enter_context(tc.tile_pool(name="smallp", bufs=8))
        actp = ctx.enter_context(tc.tile_pool(name="actp", bufs=1))
        pp = ctx.enter_context(tc.tile_pool(name="pp", bufs=4))
        gp = ctx.enter_context(tc.tile_pool(name="gp", bufs=4))
        dramp = ctx.enter_context(tc.tile_pool(name="dramp", bufs=3, space="DRAM"))
        wpp = ctx.enter_context(tc.tile_pool(name="wpp", bufs=4))
        lgp = ctx.enter_context(tc.tile_pool(name="lgp", bufs=4))

        # ================= EMBED =================
        with tc.tile_pool(name="xp", bufs=2) as xp, \
             tc.tile_pool(name="wembp", bufs=2) as wep:
            emb_ps = [tmps.tile([128, D], F32, name=f"embps{mt}", tag=f"tm{mt}")
                      for mt in range(TP)]
            for ch in range(VP // XCH):
                x_t = xp.tile([128, XCH, T], BF16, name="x_t", tag="x")
                nc.sync.dma_start(
                    x_t[:], xT.ap()[ch * XCH * 128:(ch + 1) * XCH * 128, :]
                    .rearrange("(k p) n -> p k n", p=128))
                if XCH == WCH:
                    w_t = wep.tile([128, WCH, D], BF16, name="w_t", tag="wemb")
                    nc.sync.dma_start(
                        w_t[:], Wemb.ap()[ch * WCH * 128:(ch + 1) * WCH * 128, :]
                        .rearrange("(k p) n -> p k n", p=128))
                for k in range(XCH):
                    kt = ch * XCH + k
                    for mt in range(TP):
                        nc.tensor.matmul(emb_ps[mt][:],
                                         x_t[:, k, ts(mt, 128)],
                                         w_t[:, k, :],
                                         start=(kt == 0), stop=(kt == VP - 1))
            for mt in range(TP):
                nc.vector.tensor_add(resid[:, mt, :], emb_ps[mt][:],
                                     peb_sb[:, mt, :])

        # ---- shared activation tiles (allocated fresh per layer via pools) ----
        def layernorm(dst_tm, src):
            for mt in range(TP):
                stats = smallp.tile([128, 6], F32, name="stats", tag="stats")
                nc.vector.bn_stats(stats[:], src[:, mt, :])
                mv = smallp.tile([128, 2], F32, name="mv", tag="mv")
                nc.vector.bn_aggr(mv[:], stats[:])
                lnv = smallp.tile([128, 1], F32, name="lnv", tag="lnv")
                nc.scalar.activation(lnv[:], mv[:, 1:2], AF.Ln,
                                     bias=eps_sb[:, 0:1])
                rstd = smallp.tile([128, 1], F32, name="rstd", tag="rstd")
                nc.scalar.activation(rstd[:], lnv[:], AF.Exp, scale=-0.5)
                nc.vector.tensor_scalar(dst_tm[:, mt, :], src[:, mt, :],
                                        mv[:, 0:1], rstd[:],
                                        op0=OP.subtract, op1=OP.mult)

        def transpose_to(dst_fm, src_tm, dst8=None):
            for ft in range(DP):
                for mt in range(TP):
                    tp_ = bigps.tile([128, 128], F32, name="trps", tag="big")
                    nc.tensor.transpose(tp_[:], src_tm[:, mt, ts(ft, 128)],
                                        eye_sb[:])
                    nc.vector.tensor_copy(dst_fm[:, ft, ts(mt, 128)], tp_[:])
                    if dst8 is not None:
                        # Pool can't read PSUM; cast from the SBUF copy
                        nc.gpsimd.tensor_copy(dst8[:, ft, ts(mt, 128)],
                                              dst_fm[:, ft, ts(mt, 128)])

        # ================= LAYERS =================
        wp_pre = []
        with tc.tile_pool(name="wqkv", bufs=2) as wqkv, \
             tc.tile_pool(name="w1p", bufs=4) as w1p, \
             tc.tile_pool(name="w2p", bufs=3) as w2p, \
             tc.tile_pool(name="smallw", bufs=2) as smallw:
            for l in range(L):
                # --- LN1 + transpose (f32r + fp8 copies) ---
                y_tm = actp.tile([128, TP, D], F32, name="y_tm", tag="y_tm",
                                 bufs=1)
                layernorm(y_tm, resid)
                y_fm = actp.tile([128, DP, T], BF16, name="y_fm", tag="y_fm",
                                 bufs=2)
                y8 = actp.tile([128, DP, T], FP8, name="y8", tag="y8", bufs=2)
                transpose_to(y_fm, y_tm, dst8=y8)

                # --- y exchange: AllGather of fp8 y (starts ASAP) ---
                cc_in = dramp.tile([YSZ], FP8, name="cc_in", tag="cc_in")
                nc.sync.dma_start(
                    cc_in[:].rearrange("(t p n) -> p t n", p=128, n=T),
                    y8[:])
                cc_out = dramp.tile([2 * YSZ], FP8, name="cc_out",
                                    tag="cc_out")
                nc.gpsimd.collective_compute(
                    "AllGather", OP.bypass, replica_groups=RG,
                    ins=[cc_in[:].opt()], outs=[cc_out[:].opt()])

                if l == L - 1:
                    # prefetch all unembed weight chunks into the last
                    # layer's collective window
                    for c in range(4):
                        wp_t = wpp.tile([128, DP, 4 * T], BF16,
                                        name="wp_t", tag="wp")
                        nc.sync.dma_start(
                            wp_t[:], Wp.ap()[:, c * 4 * T:(c + 1) * 4 * T]
                            .rearrange("(t p) n -> p t n", p=128))
                        bp_t = wpp.tile([1, 4 * T], BF16, name="bp_t",
                                        tag="bp")
                        dmaR(bp_t[:], bp.ap()[c * 4 * T:(c + 1) * 4 * T]
                             .rearrange("(o n) -> o n", o=1))
                        wp_pre.append((wp_t, bp_t))

                # --- weight loads ---
                wq_sb = wqkv.tile([128, DP, D], BF16, name="wq_sb", tag="wq")
                dmaR(wq_sb[:], Wq.ap()[l * D:(l + 1) * D, :]
                     .rearrange("(k p) o -> p k o", p=128))
                wk_sb = wqkv.tile([128, DP, D], BF16, name="wk_sb", tag="wk")
                dmaR(wk_sb[:], Wk.ap()[l * D:(l + 1) * D, :]
                     .rearrange("(k p) o -> p k o", p=128))
                wv_sb = wqkv.tile([128, DP, D], BF16, name="wv_sb", tag="wv")
                dmaR(wv_sb[:], Wv.ap()[l * D:(l + 1) * D, :]
                     .rearrange("(k p) o -> p k o", p=128))
                wo_sb = wqkv.tile([128, DP, D], BF16, name="wo_sb", tag="wo")
                dmaR(wo_sb[:], Wo.ap()[l * D:(l + 1) * D, :]
                     .rearrange("(k p) o -> p k o", p=128))
                wksel8 = wqkv.tile([128, 2 * DP, D], FP8, name="wksel8",
                                   tag="wksel")
                nc.sync.dma_start(
                    wksel8[:], Wksel.ap()[l * 128:(l + 1) * 128, :]
                    .rearrange("p (k o) -> p k o", k=2 * DP))
                wvsel8 = wqkv.tile([128, 2 * DP, D], FP8, name="wvsel8",
                                   tag="wvsel")
                nc.sync.dma_start(
                    wvsel8[:], Wvsel.ap()[l * 128:(l + 1) * 128, :]
                    .rearrange("p (k o) -> p k o", k=2 * DP))
                bq_sb = smallw.tile([128, DP], F32, name="bq_sb", tag="bq")
                nc.sync.dma_start(bq_sb[:], bqc.ap()[l * D:(l + 1) * D]
                                  .rearrange("(t p) -> p t", p=128))
                bk_sb = smallw.tile([128, DP], F32, name="bk_sb", tag="bk")
                nc.sync.dma_start(bk_sb[:], bkc.ap()[l * D:(l + 1) * D]
                                  .rearrange("(t p) -> p t", p=128))
                r4_sb = smallw.tile([1, 4, D], BF16, name="r4_sb", tag="r4")
                dmaR(r4_sb[:], rows4.ap()[l:l + 1, :]
                     .rearrange("o (r d) -> o r d", r=4))
                b1_sb = smallw.tile([128, FP], F32, name="b1_sb", tag="b1")
                nc.sync.dma_start(b1_sb[:], b1c.ap()[l * F:(l + 1) * F]
                                  .rearrange("(t p) -> p t", p=128))

                # --- local K, V, Q projections (full-precision y) ---
                k_local = actp.tile([128, DP, T], FP8, name="k_local",
                                    tag="k_local")
                for t in range(DP):
                    psk = bigps.tile([128, T], F32, name="psk", tag="big")
                    for kt in range(DP):
                        nc.tensor.matmul(psk[:], wk_sb[:, kt, ts(t, 128)],
                                         y_fm[:, kt, :],
                                         start=(kt == 0), stop=(kt == DP - 1))
                    nc.scalar.activation(k_local[:, t, :], psk[:], AF.Identity,
                                         bias=bk_sb[:, t:t + 1])
                v_local = actp.tile([128, TP, H, EP], FP8, name="v_local",
                                    tag="v_local")
                nc.vector.memset(v_local[:, :, :, E:E + 1], 1.0)
                for mt in range(TP):
                    psv = bigps.tile([128, D], F32, name="psv", tag="big")
                    for kt in range(DP):
                        nc.tensor.matmul(psv[:], y_fm[:, kt, ts(mt, 128)],
                                         wv_sb[:, kt, :],
                                         start=(kt == 0), stop=False)
                    nc.tensor.matmul(psv[:], ones_sb[0:1, :],
                                     r4_sb[0:1, 0, :], start=False, stop=True)
                    nc.vector.tensor_copy(
                        v_local[:, mt, :, 0:E],
                        psv[:].rearrange("p (h e) -> p h e", h=H))
                q_fm = actp.tile([128, DP, T], FP8, name="q_fm", tag="q_fm")
                for t in range(DP):
                    psq = bigps.tile([128, T], F32, name="psq", tag="big")
                    for kt in range(DP):
                        nc.tensor.matmul(psq[:], wq_sb[:, kt, ts(t, 128)],
                                         y_fm[:, kt, :],
                                         start=(kt == 0), stop=(kt == DP - 1))
                    nc.vector.tensor_scalar_add(q_fm[:, t, :], psq[:],
                                                bq_sb[:, t:t + 1])

                # --- LOCAL attention (overlaps the collective) ---
                # head pairs run interleaved on two PSUM banks so the ACT
                # exps of one head hide behind the other head's matmuls
                o_loc = [actp.tile([EP, T], BF16, name=f"o_loc{h}",
                                   tag=f"o_loc{h}") for h in range(H)]
                for hp in range(H // 2):
                    h0, h1 = 2 * hp, 2 * hp + 1
                    ps = []
                    ps.append(ops_.tile([EP, T], F32, name="o_ps", tag="o"))
                    ps.append(tmps.tile([EP, T], F32, name="o_ps3",
                                        tag="tm3"))
                    for m in range(TP):
                        pts = []
                        for i, h in enumerate((h0, h1)):
                            po, pt = (h % 2) * E, h // 2
                            sc = bigps.tile([128, T], F32, name="sc",
                                            tag="big")
                            nc.tensor.matmul(
                                sc[:], k_local[po:po + E, pt, ts(m, 128)],
                                q_fm[po:po + E, pt, :],
                                start=True, stop=True)
                            p_t = pp.tile([128, T], FP8, name="p_t", tag="p")
                            nc.scalar.activation(p_t[:], sc[:], AF.Exp)
                            pts.append(p_t)
                        for i, h in enumerate((h0, h1)):
                            nc.tensor.matmul(ps[i][:], v_local[:, m, h, :],
                                             pts[i][:], start=(m == 0),
                                             stop=(m == TP - 1))
                    nc.vector.tensor_copy(o_loc[h0][:], ps[0][:])
                    nc.vector.tensor_copy(o_loc[h1][:], ps[1][:])

                # --- keep-warm: PE runs dry ~9us before the collective
                # lands; idle >3us drops the modeled PE clock to mid-state
                # for the first 3us of the remote phase. A chained block of
                # throwaway matmuls pins the ramp (and fills the gap).
                warm_ps = bigps.tile([128, T], F32, name="warm", tag="big")
                for w in range(40):
                    nc.tensor.matmul(warm_ps[:], wk_sb[:, 0, ts(0, 128)],
                                     y_fm[:, 0, :], start=True, stop=True)

                # --- gathered y in (both halves) ---
                yg = actp.tile([128, 2 * DP, T], FP8, name="yg", tag="yg")
                for r in range(2):
                    nc.sync.dma_start(
                        yg[:, r * DP:(r + 1) * DP, :],
                        cc_out[r * YSZ:(r + 1) * YSZ]
                        .rearrange("(t p n) -> p t n", p=128, n=T))

                # --- remote K/V via per-core selection weights ---
                k_rem = actp.tile([128, DP, T], FP8, name="k_rem",
                                  tag="k_rem")
                for t in range(DP):
                    pskr = bigps.tile([128, T], F32, name="pskr", tag="big")
                    for kt in range(2 * DP):
                        nc.tensor.matmul(pskr[:], wksel8[:, kt, ts(t, 128)],
                                         yg[:, kt, :],
                                         start=(kt == 0),
                                         stop=(kt == 2 * DP - 1))
                    nc.scalar.activation(k_rem[:, t, :], pskr[:], AF.Identity,
                                         bias=bk_sb[:, t:t + 1],
                                         scale=1.0 / SSEL)
                v_rem = actp.tile([128, TP, H, EP], FP8, name="v_rem",
                                  tag="v_rem")
                nc.vector.memset(v_rem[:, :, :, E:E + 1], 1.0)
                for mt in range(TP):
                    psvr = bigps.tile([128, D], F32, name="psvr", tag="big")
                    for kt in range(2 * DP):
                        nc.tensor.matmul(psvr[:], yg[:, kt, ts(mt, 128)],
                                         wvsel8[:, kt, :],
                                         start=(kt == 0), stop=False)
                    nc.tensor.matmul(psvr[:], ones_sb[0:1, :],
                                     r4_sb[0:1, 3, :], start=False, stop=True)
                    nc.scalar.activation(
                        v_rem[:, mt, :, 0:E],
                        psvr[:].rearrange("p (h e) -> p h e", h=H),
                        AF.Copy, scale=1.0 / SSEL)

                # --- REMOTE attention + merge ---
                # o_ps alternates between two banks (tm3 is attention-idle)
                # and each head's merge is emitted one head late, so the
                # merge's DVE chain never head-of-line blocks the PE queue.
                o_fm = actp.tile([128, DP, T], BF16, name="o_fm", tag="o_fm")

                def merge_head(h, o_ps):
                    po, pt = (h % 2) * E, h // 2
                    den_f = smallp.tile([1, T], F32, name="den_f",
                                        tag="den_f", bufs=2)
                    nc.vector.tensor_add(den_f[:], o_loc[h][E:E + 1, :],
                                         o_ps[E:E + 1, :])
                    recip_f = smallp.tile([1, T], F32, name="recip_f",
                                          tag="recip_f", bufs=2)
                    nc.vector.reciprocal(recip_f[:], den_f[:])
                    recip_r = smallp.tile([1, T], BF16, name="recip_r",
                                          tag="recip_r", bufs=2)
                    nc.scalar.activation(recip_r[:], recip_f[:], AF.Copy)
                    bc_ps = bigps.tile([E, T], F32, name="bc_ps", tag="big")
                    nc.tensor.matmul(bc_ps[:], ones_b16[0:1, 0:E], recip_r[:],
                                     start=True, stop=True)
                    bc_sb = pp.tile([E, T], F32, name="bc_sb", tag="bc",
                                    bufs=2)
                    nc.vector.tensor_copy(bc_sb[:], bc_ps[:])
                    o_t = pp.tile([E, T], F32, name="o_t", tag="o_t", bufs=2)
                    nc.vector.tensor_add(o_t[:], o_loc[h][0:E, :],
                                         o_ps[0:E, :])
                    nc.vector.tensor_mul(o_fm[po:po + E, pt, :],
                                         o_t[:], bc_sb[:])

                prev_merge = None
                for h in range(H):
                    po, pt = (h % 2) * E, h // 2
                    if h % 2 == 0:
                        o_ps = ops_.tile([EP, T], F32, name="o_ps", tag="o")
                    else:
                        o_ps = tmps.tile([EP, T], F32, name="o_ps3",
                                         tag="tm3")
                    for m in range(TP):
                        sc = bigps.tile([128, T], F32, name="sc", tag="big")
                        nc.tensor.matmul(sc[:],
                                         k_rem[po:po + E, pt, ts(m, 128)],
                                         q_fm[po:po + E, pt, :],
                                         start=True, stop=True)
                        p_t = pp.tile([128, T], FP8, name="p_t", tag="p")
                        nc.scalar.activation(p_t[:], sc[:], AF.Exp)
                        nc.tensor.matmul(o_ps[:], v_rem[:, m, h, :],
                                         p_t[:], start=(m == 0),
                                         stop=(m == TP - 1))
                    if prev_merge is not None:
                        merge_head(*prev_merge)
                    prev_merge = (h, o_ps)
                merge_head(*prev_merge)

                # --- Wo + residual ---
                for mt in range(TP):
                    pso = tmps.tile([128, D], F32, name="pso", tag=f"tm{mt}")
                    for kt in range(DP):
                        nc.tensor.matmul(pso[:], o_fm[:, kt, ts(mt, 128)],
                                         wo_sb[:, kt, :],
                                         start=(kt == 0), stop=False)
                    nc.tensor.matmul(pso[:], ones_sb[0:1, :],
                                     r4_sb[0:1, 1, :], start=False, stop=True)
                    nc.vector.tensor_add(resid[:, mt, :], resid[:, mt, :],
                                         pso[:])

                # --- LN2 + transpose ---
                y_tm2 = actp.tile([128, TP, D], F32, name="y_tm2", tag="y_tm")
                layernorm(y_tm2, resid)
                y2_fm = actp.tile([128, DP, T], BF16, name="y2_fm",
                                  tag="y_fm", bufs=2)
                transpose_to(y2_fm, y_tm2)

                # --- FFN (streamed: FFN1 tile -> gelu -> FFN2 partial) ---
                w1_t = []
                for kt in range(DP):
                    w1k = w1p.tile([128, F], BF16, name="w1k", tag="w1")
                    dmaR(w1k[:], W1.ap()[l * D + kt * 128:
                                         l * D + (kt + 1) * 128, :])
                    w1_t.append(w1k)
                f2ps = [tmps.tile([128, D], F32, name=f"f2ps{mt}",
                                  tag=f"tm{mt}") for mt in range(TP)]
                for ft in range(FP):
                    if ft % 4 == 0:
                        w2c = w2p.tile([128, 4, D], BF16, name="w2c", tag="w2")
                        dmaR(w2c[:], W2.ap()[l * F + (ft // 4) * 512:
                                             l * F + (ft // 4 + 1) * 512, :]
                             .rearrange("(k p) o -> p k o", p=128))
                    psf = bigps.tile([128, T], F32, name="psf", tag="big")
                    for kt in range(DP):
                        nc.tensor.matmul(psf[:], w1_t[kt][:, ts(ft, 128)],
                                         y2_fm[:, kt, :],
                                         start=(kt == 0), stop=(kt == DP - 1))
                    g_t = gp.tile([128, T], BF16, name="g_t", tag="g")
                    nc.scalar.activation(g_t[:], psf[:], AF.Gelu,
                                         bias=b1_sb[:, ft:ft + 1])
                    for mt in range(TP):
                        nc.tensor.matmul(f2ps[mt][:], g_t[:, ts(mt, 128)],
                                         w2c[:, ft % 4, :],
                                         start=(ft == 0), stop=False)
                for mt in range(TP):
                    nc.tensor.matmul(f2ps[mt][:], ones_sb[0:1, :],
                                     r4_sb[0:1, 2, :], start=False, stop=True)
                    nc.vector.tensor_add(resid[:, mt, :], resid[:, mt, :],
                                         f2ps[mt][:])

        # ================= FINAL LN + UNEMBED =================
        lnf_tm = actp.tile([128, TP, D], F32, name="lnf_tm", tag="y_tm")
        layernorm(lnf_tm, resid)
        lnf_fm = actp.tile([128, DP, T], BF16, name="lnf_fm", tag="lnf_fm")
        for ft in range(DP):
            for mt in range(TP):
                tp_ = bigps.tile([128, 128], F32, name="trps", tag="big")
                nc.tensor.transpose(tp_[:], lnf_tm[:, mt, ts(ft, 128)],
                                    eye_sb[:])
                nc.vector.tensor_copy(lnf_fm[:, ft, ts(mt, 128)], tp_[:])

        if True:
            for c in range(V // (4 * T)):
                if c < len(wp_pre):
                    wp_t, bp_t = wp_pre[c]
                else:
                    wp_t = wpp.tile([128, DP, 4 * T], BF16, name="wp_t",
                                    tag="wp")
                    nc.sync.dma_start(
                        wp_t[:], Wp.ap()[:, c * 4 * T:(c + 1) * 4 * T]
                        .rearrange("(t p) n -> p t n", p=128))
                    bp_t = wpp.tile([1, 4 * T], BF16, name="bp_t", tag="bp")
                    dmaR(bp_t[:], bp.ap()[c * 4 * T:(c + 1) * 4 * T]
                         .rearrange("(o n) -> o n", o=1))
                for half in range(4):
                    vc = c * 4 + half
                    for mt in range(TP):
                        psl = bigps.tile([128, T], F32, name="psl", tag="big")
                        for kt in range(DP):
                            nc.tensor.matmul(psl[:],
                                             lnf_fm[:, kt, ts(mt, 128)],
                                             wp_t[:, kt,
                                                  half * T:(half + 1) * T],
                                             start=(kt == 0), stop=False)
                        nc.tensor.matmul(psl[:], ones_sb[0:1, :],
                                         bp_t[0:1, half * T:(half + 1) * T],
                                         start=False, stop=True)
                        lg = lgp.tile([128, T], F32, name="lg", tag="lg")
                        if (vc * TP + mt) % 2 == 0:
                            nc.vector.tensor_copy(lg[:], psl[:])
                        else:
                            nc.scalar.activation(lg[:], psl[:], AF.Copy)
                        nc.sync.dma_start(
                            logits.ap()[mt * 128:(mt + 1) * 128,
                                        vc * T:(vc + 1) * T], lg[:])
        ctx.close()

    nc.compile()
    return nc


def _prep_inputs(inputs):
    f = {k: np.asarray(v, dtype=np.float32) for k, v in inputs.items()}
    x, Wemb_, bemb = f["x"], f["Wemb"], f["bemb"]
    scale = E ** -0.5
    Wq_p = np.empty((L, D, D), np.float32)
    Wk_p = np.empty((L, D, D), np.float32)
    Wv_p = np.empty((L, D, D), np.float32)
    bq_p = np.empty((L, D), np.float32)
    bk_p = np.empty((L, D), np.float32)
    rows4 = np.empty((L, 4, D), np.float32)
    W1_p = np.empty((L, D, F), np.float32)
    b1_p = np.empty((L, F), np.float32)
    for l in range(L):
        g1, b1l = f["ln1_g"][l], f["ln1_b"][l]
        Wq_l = f["Wq"][l].transpose(1, 0, 2).reshape(D, D)
        Wk_l = f["Wk"][l].transpose(1, 0, 2).reshape(D, D)
        Wv_l = f["Wv"][l].transpose(1, 0, 2).reshape(D, D)
        Wq_p[l] = (g1[:, None] * Wq_l) * scale
        bq_p[l] = (b1l @ Wq_l + f["bq"][l].reshape(-1)) * scale
        Wk_p[l] = g1[:, None] * Wk_l
        bk_p[l] = b1l @ Wk_l + f["bk"][l].reshape(-1)
        Wv_p[l] = g1[:, None] * Wv_l
        rows4[l, 0] = b1l @ Wv_l + f["bv"][l].reshape(-1)
        rows4[l, 1] = f["bo"][l]
        g2, b2l = f["ln2_g"][l], f["ln2_b"][l]
        W1_p[l] = g2[:, None] * f["W1"][l]
        b1_p[l] = b2l @ f["W1"][l] + f["b1"][l]
        rows4[l, 2] = f["b2"][l]
        rows4[l, 3] = SSEL * rows4[l, 0]
    Wp_p = f["lnf_g"][:, None] * f["Wp"]
    bp_p = f["lnf_b"] @ f["Wp"] + f["bp"]
    pe = _positional_encoding(NTOK, D)

    bf16 = ml_dtypes.bfloat16
    fp8 = ml_dtypes.float8_e4m3
    shared = {
        "eye": np.eye(128, dtype=np.float32),
        "onesv": np.ones((1, 128), bf16),
        "Wemb": np.ascontiguousarray(Wemb_.astype(bf16)),
        "Wq": np.ascontiguousarray(Wq_p.reshape(L * D, D).astype(bf16)),
        "Wk": np.ascontiguousarray(Wk_p.reshape(L * D, D).astype(bf16)),
        "Wv": np.ascontiguousarray(Wv_p.reshape(L * D, D).astype(bf16)),
        "Wo": np.ascontiguousarray(f["Wo"].reshape(L * D, D).astype(bf16)),
        "bqc": np.ascontiguousarray(bq_p.reshape(L * D)),
        "bkc": np.ascontiguousarray(bk_p.reshape(L * D)),
        "rows4": np.ascontiguousarray(rows4.reshape(L, 4 * D).astype(bf16)),
        "W1": np.ascontiguousarray(W1_p.reshape(L * D, F).astype(bf16)),
        "b1c": np.ascontiguousarray(b1_p.reshape(L * F)),
        "W2": np.ascontiguousarray(f["W2"].reshape(L * F, D).astype(bf16)),
        "Wp": np.ascontiguousarray(Wp_p.astype(bf16)),
        "bp": np.ascontiguousarray(bp_p.astype(bf16)),
    }
    in_maps = []
    for c in range(NCORES):
        bb, hh = c // 2, c % 2
        n0 = hh * T
        m = dict(shared)
        m["xT"] = np.ascontiguousarray(x[bb, n0:n0 + T, :].T.astype(bf16))
        m["peb"] = np.ascontiguousarray(pe[n0:n0 + T] + bemb)
        ksel = np.concatenate([Wk_p * (SSEL * hh), Wk_p * (SSEL * (1 - hh))],
                              axis=1)
        vsel = np.concatenate([Wv_p * (SSEL * hh), Wv_p * (SSEL * (1 - hh))],
                              axis=1)
        # per-partition-contiguous layout: [L,128,6*D] so each partition's
        # 2304B arrives as one descriptor (<512B elems pay 2x in the DMA)
        m["Wksel"] = np.ascontiguousarray(
            ksel.reshape(L, 6, 128, D).transpose(0, 2, 1, 3)
            .reshape(L * 128, 6 * D).astype(fp8))
        m["Wvsel"] = np.ascontiguousarray(
            vsel.reshape(L, 6, 128, D).transpose(0, 2, 1, 3)
            .reshape(L * 128, 6 * D).astype(fp8))
        in_maps.append(m)
    return in_maps


_NC_CACHE = []


def kernel(**inputs):
    import time
    from concourse.bass_utils import run_bass_kernel_spmd

    in_maps = _prep_inputs(inputs)
    if not _NC_CACHE:
        _NC_CACHE.append(build_nc())
    nc = _NC_CACHE[0]
    t0 = time.time()
    res = run_bass_kernel_spmd(nc, in_maps, core_ids=list(range(NCORES)))
    t1 = time.time()
    print(f"[kernel] run_bass_kernel_spmd wall: {(t1 - t0) * 1e3:.1f} ms",
          file=sys.stderr)
    out = np.empty((B, NTOK, V), np.float32)
    for c in range(NCORES):
        out[c // 2, (c % 2) * T:(c % 2) * T + T, :] = res.results[c]["logits"]
    return out
